# revision 25
# baseline (speedup 1.0000x reference)
"""Trainium2 Bass kernel for nn_AttnResBase (layer-axis softmax attention).

Math (see reference):
    qW      = query.reshape(-1) @ W_key                      # [H]
    scores  = einsum('lbsh,h->bsl', preceding, qW) / sqrt(H)
    w       = softmax(scores, axis=-1)                       # over L
    out     = einsum('bsl,lbsh->bsh', w, preceding)

`current_output` is unused by the math. The problem is strongly memory
bound: preceding is 8*4*4096*768 fp32 = 403 MB, read once; output 50 MB.

Distribution: flatten (b, s) -> N = 16384 rows; each of the 8 cores gets a
contiguous block of 2048 rows (no cross-device communication needed). qW is
tiny: computed on host, pre-scaled by 1/sqrt(H), replicated to all cores.

Per-core kernel (rows processed in 16 tiles of 128 = partition dim):
  - 2 merged DMAs load the tile's 8 layer slices       2x [128, 4, 768]
  - scores: DVE fused scalar_tensor_tensor per layer   accum -> [128, 8]
  - softmax: ACT exp (+denominator via accum_out), DVE reciprocal,
    DVE per-partition scale                            -> w [128, 8]
    (no max-subtraction: scores ~ N(0, 0.02), exp is exact-safe)
  - weighted sum: for each layer, build D_l = diag(w[:, l]) via ACT
    (identity * per-partition scalar), then PE matmul D_l @ prec_l
    accumulating over layers in PSUM (float32r = full-rate fp32 matmul,
    hw rounds operands to ~TF32: measured ~2e-4 output rel err)
  - copy PSUM -> SBUF on ACT, DMA out via the ACT HWDGE queue
"""

import sys
import math
import numpy as np
from contextlib import ExitStack

for _p in ("/opt/trn_rl_repo", "/root/.axon_site/_ro/trn_rl_repo"):
    if _p not in sys.path:
        sys.path.append(_p)

import concourse.bass as bass
import concourse.bacc as bacc
import concourse.tile as tile
from concourse import mybir
from concourse.bass_utils import run_bass_kernel_spmd

F32 = mybir.dt.float32
F32R = mybir.dt.float32r
ALU = mybir.AluOpType
ACTF = mybir.ActivationFunctionType

B, S, H, L = 4, 4096, 768, 8
N_CORES = 8
N_ROWS_TOTAL = B * S
ROWS_PER_CORE = N_ROWS_TOTAL // N_CORES  # 2048
TILE_ROWS = 128
LH = L // 2  # layers per half-tile


def build_nc(n_rows: int = ROWS_PER_CORE) -> bass.Bass:
    nc = bacc.Bacc("TRN2", target_bir_lowering=False, debug=False)
    # prec is float32r so the PE can matmul fp32 bits at full rate
    # (1 cycle/row vs 4 for plain fp32). DVE consumers bitcast back to
    # plain f32 (same bits).
    prec = nc.declare_dram_parameter("prec", [L, n_rows, H], F32R, isOutput=False)
    # consts: [:, 0:768] = qW/sqrt(H) replicated, [:, 768:896] = identity
    consts = nc.declare_dram_parameter("consts", [128, H + 128], F32, isOutput=False)
    out = nc.declare_dram_parameter("out", [n_rows, H], F32, isOutput=True)

    n_tiles = n_rows // TILE_ROWS
    with tile.TileContext(nc) as tc, ExitStack() as ctx:
        cpool = ctx.enter_context(tc.tile_pool(name="const", bufs=1))
        ppool = ctx.enter_context(tc.tile_pool(name="prec", bufs=3))
        jpool = ctx.enter_context(tc.tile_pool(name="junk", bufs=2))
        spool = ctx.enter_context(tc.tile_pool(name="small", bufs=2))
        dpool = ctx.enter_context(tc.tile_pool(name="diag", bufs=4))
        opool = ctx.enter_context(tc.tile_pool(name="osb", bufs=2))
        qpool = ctx.enter_context(
            tc.tile_pool(name="psum", bufs=2, space=bass.MemorySpace.PSUM)
        )

        csb = cpool.tile([128, H + 128], F32, tag="consts")
        nc.sync.dma_start(out=csb[:], in_=consts[:])
        qw_sb = csb[:, 0:H]
        id_sb = csb[:, H : H + 128]

        for t in range(n_tiles):
            r0 = t * TILE_ROWS
            # two merged DMAs per tile: scores for layers 0-3 can start
            # while layers 4-7 are still in flight
            halves = []
            for hlf, tag in ((0, "pa"), (1, "pb")):
                pt = ppool.tile([TILE_ROWS, LH, H], F32R, tag=tag)
                nc.sync.dma_start(
                    out=pt[:],
                    in_=prec[
                        hlf * LH : (hlf + 1) * LH, r0 : r0 + TILE_ROWS, :
                    ].rearrange("l r h -> r l h"),
                )
                halves.append(pt)

            # Per layer: score s_l (DVE fused dot), e_l = exp(s_l) (ACT),
            # D_l = diag(e_l) (ACT), then PE accumulates the UNNORMALIZED
            # weighted sum in PSUM. The softmax denominator is folded into
            # the PSUM->SBUF copy as a per-partition 1/sum(e) scale, so PE
            # work for layer l starts right after its score - no softmax
            # barrier across all 8 layers.
            expw = spool.tile([TILE_ROWS, L], F32, tag="expw")
            junk = jpool.tile([TILE_ROWS, H], F32, tag="junk")
            po = qpool.tile([TILE_ROWS, H], F32, tag="po")
            for l in range(L):
                s_l = spool.tile([TILE_ROWS, 1], F32, tag=f"s{l}")
                nc.vector.scalar_tensor_tensor(
                    out=junk[:],
                    in0=halves[l // LH][:, l % LH, :].bitcast(F32),
                    scalar=1.0,
                    in1=qw_sb,
                    op0=ALU.mult,
                    op1=ALU.mult,
                    accum_out=s_l[:],
                )
                # scores ~ N(0, 0.02): exp without max-subtraction is safe
                nc.scalar.activation(out=expw[:, l : l + 1], in_=s_l[:], func=ACTF.Exp)
                dql = dpool.tile([TILE_ROWS, 128], F32R, tag="dql")
                nc.scalar.mul(dql[:], id_sb, expw[:, l : l + 1])
                rhs = halves[l // LH]
                nc.tensor.matmul(
                    po[:, 0:512],
                    dql[:],
                    rhs[:, l % LH, 0:512],
                    start=(l == 0),
                    stop=(l == L - 1),
                )
                nc.tensor.matmul(
                    po[:, 512:H],
                    dql[:],
                    rhs[:, l % LH, 512:H],
                    start=(l == 0),
                    stop=(l == L - 1),
                )

            denom = spool.tile([TILE_ROWS, 1], F32, tag="denom")
            nc.vector.tensor_reduce(
                out=denom[:], in_=expw[:], axis=mybir.AxisListType.X, op=ALU.add
            )
            recip = spool.tile([TILE_ROWS, 1], F32, tag="recip")
            nc.vector.reciprocal(recip[:], denom[:])

            osb = opool.tile([TILE_ROWS, H], F32, tag="osb")
            # normalize during the PSUM->SBUF copy (free: scale is per-partition)
            nc.scalar.mul(osb[:], po[:], recip[:, 0:1])
            # store via the ACT HWDGE queue so it doesn't serialize with loads
            nc.scalar.dma_start(out=out[r0 : r0 + TILE_ROWS, :], in_=osb[:])

    nc.compile()
    return nc


def _prep_inputs(current_output, preceding, W_key, query):
    """Host-side prep: qW projection, per-core shards."""
    q = np.asarray(query, dtype=np.float32).reshape(-1)
    w_key = np.asarray(W_key, dtype=np.float32)
    qw = (q @ w_key) / np.float32(math.sqrt(H))
    qw_rep = np.broadcast_to(qw[None, :], (128, H))
    consts = np.ascontiguousarray(
        np.concatenate([qw_rep, np.eye(128, dtype=np.float32)], axis=1)
    )

    prec = np.asarray(preceding, dtype=np.float32).reshape(L, N_ROWS_TOTAL, H)
    in_maps = []
    for c in range(N_CORES):
        r0 = c * ROWS_PER_CORE
        shard = np.ascontiguousarray(prec[:, r0 : r0 + ROWS_PER_CORE, :])
        in_maps.append({"prec": shard, "consts": consts})
    return in_maps


_NC_CACHE = {}


def _get_nc():
    if "nc" not in _NC_CACHE:
        _NC_CACHE["nc"] = build_nc()
    return _NC_CACHE["nc"]


def kernel(current_output, preceding, W_key, query, _trace=False):
    in_maps = _prep_inputs(current_output, preceding, W_key, query)
    nc = _get_nc()
    res = run_bass_kernel_spmd(
        nc, in_maps, core_ids=list(range(N_CORES)), trace=_trace
    )
    outs = [res.results[c]["out"] for c in range(N_CORES)]
    full = np.concatenate(outs, axis=0).reshape(B, S, H)
    if _trace:
        return full, res
    return full


# revision 29
# speedup vs baseline: 1.0213x; 1.0213x over previous
"""Trainium2 Bass kernel for nn_AttnResBase (layer-axis softmax attention).

Math (see reference):
    qW      = query.reshape(-1) @ W_key                      # [H]
    scores  = einsum('lbsh,h->bsl', preceding, qW) / sqrt(H)
    w       = softmax(scores, axis=-1)                       # over L
    out     = einsum('bsl,lbsh->bsh', w, preceding)

`current_output` is unused by the math. The problem is strongly memory
bound: preceding is 8*4*4096*768 fp32 = 403 MB, read once; output 50 MB.

Distribution: flatten (b, s) -> N = 16384 rows; each of the 8 cores gets a
contiguous block of 2048 rows (no cross-device communication needed). qW is
tiny: computed on host, pre-scaled by 1/sqrt(H), replicated to all cores.

Per-core kernel (rows processed in 16 tiles of 128 = partition dim):
  - 2 merged DMAs load the tile's 8 layer slices       2x [128, 4, 768]
  - scores: DVE fused scalar_tensor_tensor per layer   accum -> [128, 8]
  - softmax: ACT exp (+denominator via accum_out), DVE reciprocal,
    DVE per-partition scale                            -> w [128, 8]
    (no max-subtraction: scores ~ N(0, 0.02), exp is exact-safe)
  - weighted sum: for each layer, build D_l = diag(w[:, l]) via ACT
    (identity * per-partition scalar), then PE matmul D_l @ prec_l
    accumulating over layers in PSUM (float32r = full-rate fp32 matmul,
    hw rounds operands to ~TF32: measured ~2e-4 output rel err)
  - copy PSUM -> SBUF on ACT, DMA out via the ACT HWDGE queue
"""

import sys
import math
import numpy as np
from contextlib import ExitStack

for _p in ("/opt/trn_rl_repo", "/root/.axon_site/_ro/trn_rl_repo"):
    if _p not in sys.path:
        sys.path.append(_p)

import concourse.bass as bass
import concourse.bacc as bacc
import concourse.tile as tile
from concourse import mybir
from concourse.bass_utils import run_bass_kernel_spmd

F32 = mybir.dt.float32
F32R = mybir.dt.float32r
ALU = mybir.AluOpType
ACTF = mybir.ActivationFunctionType

B, S, H, L = 4, 4096, 768, 8
N_CORES = 8
N_ROWS_TOTAL = B * S
ROWS_PER_CORE = N_ROWS_TOTAL // N_CORES  # 2048
TILE_ROWS = 128
N_SPLIT = 4  # load DMAs per tile
LH = L // N_SPLIT  # layers per split


def build_nc(n_rows: int = ROWS_PER_CORE) -> bass.Bass:
    nc = bacc.Bacc("TRN2", target_bir_lowering=False, debug=False)
    # prec is float32r so the PE can matmul fp32 bits at full rate
    # (1 cycle/row vs 4 for plain fp32). DVE consumers bitcast back to
    # plain f32 (same bits).
    prec = nc.declare_dram_parameter("prec", [L, n_rows, H], F32R, isOutput=False)
    # consts: [:, 0:768] = qW/sqrt(H) replicated, [:, 768:896] = identity
    consts = nc.declare_dram_parameter("consts", [128, H + 128], F32, isOutput=False)
    out = nc.declare_dram_parameter("out", [n_rows, H], F32, isOutput=True)

    n_tiles = n_rows // TILE_ROWS
    with tile.TileContext(nc) as tc, ExitStack() as ctx:
        cpool = ctx.enter_context(tc.tile_pool(name="const", bufs=1))
        ppool = ctx.enter_context(tc.tile_pool(name="prec", bufs=4))
        jpool = ctx.enter_context(tc.tile_pool(name="junk", bufs=2))
        spool = ctx.enter_context(tc.tile_pool(name="small", bufs=2))
        dpool = ctx.enter_context(tc.tile_pool(name="diag", bufs=4))
        opool = ctx.enter_context(tc.tile_pool(name="osb", bufs=2))
        qpool = ctx.enter_context(
            tc.tile_pool(name="psum", bufs=3, space=bass.MemorySpace.PSUM)
        )

        csb = cpool.tile([128, H + 128], F32, tag="consts")
        nc.sync.dma_start(out=csb[:], in_=consts[:])
        qw_sb = csb[:, 0:H]
        id_sb = csb[:, H : H + 128]

        for t in range(n_tiles):
            r0 = t * TILE_ROWS
            # split loads: scores for early layers start while later
            # layers are still in flight
            halves = []
            for hlf in range(N_SPLIT):
                pt = ppool.tile([TILE_ROWS, LH, H], F32R, tag=f"p{hlf}")
                nc.sync.dma_start(
                    out=pt[:],
                    in_=prec[
                        hlf * LH : (hlf + 1) * LH, r0 : r0 + TILE_ROWS, :
                    ].rearrange("l r h -> r l h"),
                )
                halves.append(pt)

            # Per layer: score s_l (DVE fused dot), e_l = exp(s_l) (ACT),
            # D_l = diag(e_l) (ACT), then PE accumulates the UNNORMALIZED
            # weighted sum in PSUM. The softmax denominator is folded into
            # the PSUM->SBUF copy as a per-partition 1/sum(e) scale, so PE
            # work for layer l starts right after its score - no softmax
            # barrier across all 8 layers.
            expw = spool.tile([TILE_ROWS, L], F32, tag="expw")
            junk = jpool.tile([TILE_ROWS, H], F32, tag="junk")
            po = qpool.tile([TILE_ROWS, H], F32, tag="po")
            for l in range(L):
                s_l = spool.tile([TILE_ROWS, 1], F32, tag=f"s{l}")
                nc.vector.scalar_tensor_tensor(
                    out=junk[:],
                    in0=halves[l // LH][:, l % LH, :].bitcast(F32),
                    scalar=1.0,
                    in1=qw_sb,
                    op0=ALU.mult,
                    op1=ALU.mult,
                    accum_out=s_l[:],
                )
                # scores ~ N(0, 0.02): exp without max-subtraction is safe
                nc.scalar.activation(out=expw[:, l : l + 1], in_=s_l[:], func=ACTF.Exp)
                dql = dpool.tile([TILE_ROWS, 128], F32R, tag="dql")
                nc.scalar.mul(dql[:], id_sb, expw[:, l : l + 1])
                rhs = halves[l // LH]
                nc.tensor.matmul(
                    po[:, 0:512],
                    dql[:],
                    rhs[:, l % LH, 0:512],
                    start=(l == 0),
                    stop=(l == L - 1),
                )
                nc.tensor.matmul(
                    po[:, 512:H],
                    dql[:],
                    rhs[:, l % LH, 512:H],
                    start=(l == 0),
                    stop=(l == L - 1),
                )

            denom = spool.tile([TILE_ROWS, 1], F32, tag="denom")
            nc.vector.tensor_reduce(
                out=denom[:], in_=expw[:], axis=mybir.AxisListType.X, op=ALU.add
            )
            recip = spool.tile([TILE_ROWS, 1], F32, tag="recip")
            nc.vector.reciprocal(recip[:], denom[:])

            osb = opool.tile([TILE_ROWS, H], F32, tag="osb")
            # normalize during the PSUM->SBUF copy (free: scale is per-partition)
            nc.scalar.mul(osb[:], po[:], recip[:, 0:1])
            # store via the ACT HWDGE queue so it doesn't serialize with loads
            nc.scalar.dma_start(out=out[r0 : r0 + TILE_ROWS, :], in_=osb[:])

    nc.compile()
    return nc


def _prep_inputs(current_output, preceding, W_key, query):
    """Host-side prep: qW projection, per-core shards."""
    q = np.asarray(query, dtype=np.float32).reshape(-1)
    w_key = np.asarray(W_key, dtype=np.float32)
    qw = (q @ w_key) / np.float32(math.sqrt(H))
    qw_rep = np.broadcast_to(qw[None, :], (128, H))
    consts = np.ascontiguousarray(
        np.concatenate([qw_rep, np.eye(128, dtype=np.float32)], axis=1)
    )

    prec = np.asarray(preceding, dtype=np.float32).reshape(L, N_ROWS_TOTAL, H)
    in_maps = []
    for c in range(N_CORES):
        r0 = c * ROWS_PER_CORE
        shard = np.ascontiguousarray(prec[:, r0 : r0 + ROWS_PER_CORE, :])
        in_maps.append({"prec": shard, "consts": consts})
    return in_maps


_NC_CACHE = {}


def _get_nc():
    if "nc" not in _NC_CACHE:
        _NC_CACHE["nc"] = build_nc()
    return _NC_CACHE["nc"]


def kernel(current_output, preceding, W_key, query, _trace=False):
    in_maps = _prep_inputs(current_output, preceding, W_key, query)
    nc = _get_nc()
    res = run_bass_kernel_spmd(
        nc, in_maps, core_ids=list(range(N_CORES)), trace=_trace
    )
    outs = [res.results[c]["out"] for c in range(N_CORES)]
    full = np.concatenate(outs, axis=0).reshape(B, S, H)
    if _trace:
        return full, res
    return full
